# revision 67
# baseline (speedup 1.0000x reference)
"""LongRangeProj Bass kernel for TRN2 (8 NeuronCores, channel-sharded).

Math: out[b,c,h,w] = max_o x[b,c,o] * exp(-(inv2rv*(fn-|rm|)^2
                                            + inv2av*wrap(theta-am)^2))
with fn/theta polar coords of pixel (h,w) around origin o; the angle term
is forced to 1 at each origin's own pixel (handled by a host-precomputed
center fixup max'd in at the end).

Per-core layout: partitions = 2 batches x 64 origins, free = 4096 pixels,
one channel per iteration (C/8 = 8 channels per core).  All heavy math in
fp16 (DVE 2x/4x modes, 1-cycle/row PE transposes):
  - angle wrap: round(theta/2pi - am/2pi) comes free from an int16-output
    tensor_scalar (float->int conversion rounds to nearest), then one
    mixed-dtype tensor_tensor subtract;
  - two Squares with fused per-partition affine on ACT (a tail slice of
    the radius square runs on DVE to balance the engines);
  - e = x*exp(-t) in one ACT Exp via per-partition bias ln(x);
  - PE transposes e into fp16 PSUM, DVE max-reduces the origins;
  - center fixup + fp32 convert in one tensor_tensor max.
Channel N's transposes/reduces are emitted after channel N+1's
elementwise ops so each in-order engine queue always has ready work.
Outputs accumulate in one SBUF tile, DMA'd per channel with
partition-contiguous descriptors.
"""

import numpy as np
from contextlib import ExitStack

B, C, NH, NW, H, W = 2, 64, 8, 8, 64, 64
STRIDE = 8
NCORES = 8
CL = C // NCORES          # channels per core
HW = H * W                # 4096
NO = NH * NW              # 64 origins
NBLK = HW // 128          # 32 pixel blocks of 128
NGRP = NBLK // 8          # 4 psum banks of 8 blocks
RSP = 3840                # rdn columns on ACT; the rest squared on DVE
TWO_PI = 2.0 * np.pi

# fp16 column layout of c16
C16_V = 0
C16_FN = HW
C16_ID = 2 * HW                 # 128-col fp16 identity (transpose rhs)
C16_F = 2 * HW + 128            # CL center-fixup blocks of 64
C16_W = 2 * HW + 128 + CL * 64

_built = {}


def _host_fields():
    """v (theta/2pi) and fn in [128, HW] fp16, origins duplicated per batch."""
    oy = np.arange(NH, dtype=np.float64) * STRIDE
    ox = np.arange(NW, dtype=np.float64) * STRIDE
    yg = np.arange(H, dtype=np.float64)
    xg = np.arange(W, dtype=np.float64)
    fy = yg[None, :] - oy[:, None]                      # [NH, H]
    fx = xg[None, :] - ox[:, None]                      # [NW, W]
    FY = np.broadcast_to(fy[:, None, :, None], (NH, NW, H, W))
    FX = np.broadcast_to(fx[None, :, None, :], (NH, NW, H, W))
    fn = np.sqrt(FX * FX + FY * FY)
    v = np.arctan2(FY, FX) / TWO_PI
    rs = lambda a: np.ascontiguousarray(
        np.tile(a.reshape(NO, HW), (2, 1)).astype(np.float16))
    return rs(v), rs(fn)


def _build_bass():
    import concourse.bass as bass
    import concourse.bacc as bacc
    import concourse.tile as tile
    import concourse.mybir as mybir

    f32 = mybir.dt.float32
    f16 = mybir.dt.float16
    i16 = mybir.dt.int16
    AF = mybir.ActivationFunctionType
    OP = mybir.AluOpType
    AX = mybir.AxisListType

    nc = bacc.Bacc("TRN2", target_bir_lowering=False)
    c16a_d = nc.dram_tensor("c16a", [128, 2 * HW], f16, kind="ExternalInput")
    c16b_d = nc.dram_tensor("c16b", [128, C16_W - 2 * HW], f16,
                            kind="ExternalInput")
    c32_d = nc.dram_tensor("c32", [128, 6 * CL], f32, kind="ExternalInput")
    out_d = nc.dram_tensor("out", [128, CL * NBLK * 2], f32,
                           kind="ExternalOutput")

    with ExitStack() as ctx:
        tc = ctx.enter_context(tile.TileContext(nc))
        cpool = ctx.enter_context(tc.tile_pool(name="const", bufs=1))
        work = ctx.enter_context(tc.tile_pool(name="work", bufs=2))
        psum = ctx.enter_context(tc.tile_pool(name="psum", bufs=8,
                                              space="PSUM"))
        outp = ctx.enter_context(tc.tile_pool(name="outp", bufs=2))

        OALL = cpool.tile([128, CL, NBLK, 2], f32, tag="OALL")
        C16A = cpool.tile([128, 2 * HW], f16, tag="C16A")
        C16B = cpool.tile([128, C16_W - 2 * HW], f16, tag="C16B")
        C32 = cpool.tile([128, 6 * CL], f32, tag="C32")
        HH = HW // 2
        nc.sync.dma_start(C32[:, :], c32_d[:, :])
        nc.sync.dma_start(C16A[:, HW : HW + HH], c16a_d[:, HW : HW + HH])
        nc.sync.dma_start(C16A[:, HW + HH : 2 * HW],
                          c16a_d[:, HW + HH : 2 * HW])
        nc.sync.dma_start(C16A[:, 0:HH], c16a_d[:, 0:HH])
        nc.sync.dma_start(C16A[:, HH:HW], c16a_d[:, HH:HW])
        nc.sync.dma_start(C16B[:, :], c16b_d[:, :])
        V = C16A[:, 0:HW]
        FN = C16A[:, HW : 2 * HW]
        ID = C16B[:, 0:128]
        FX = C16B[:, 128 : 128 + CL * 64]
        A2 = C32[:, 0 * CL : 1 * CL]     # -am/2pi
        S2 = C32[:, 1 * CL : 2 * CL]     # 2pi*sqrt(inv2av)
        B2 = C32[:, 2 * CL : 3 * CL]     # -s2 * am/2pi
        SR = C32[:, 3 * CL : 4 * CL]     # sqrt(inv2rv)
        BR = C32[:, 4 * CL : 5 * CL]     # -|rm| * sqrt(inv2rv)
        LX = C32[:, 5 * CL : 6 * CL]     # ln(max(x, 1e-30))

        def emit_elementwise(it):
            a2 = A2[:, it : it + 1]
            s2 = S2[:, it : it + 1]
            b2 = B2[:, it : it + 1]
            sr = SR[:, it : it + 1]
            br = BR[:, it : it + 1]
            lx = LX[:, it : it + 1]
            # rdn = (sr*fn - sr*|rm|)^2 = inv2rv * (fn-|rm|)^2
            # ACT takes [0:RSP], DVE squares the tail slice to balance load
            # (channel 0's FN/V consumers split in halves so compute starts
            # as soon as the first half-DMA lands)
            rdn = work.tile([128, HW], f16, tag="rdn")
            if it == 0:
                nc.scalar.activation(rdn[:, 0:HH], FN[:, 0:HH], AF.Square,
                                     scale=sr, bias=br)
                nc.scalar.activation(rdn[:, HH:RSP], FN[:, HH:RSP], AF.Square,
                                     scale=sr, bias=br)
            else:
                nc.scalar.activation(rdn[:, 0:RSP], FN[:, 0:RSP], AF.Square,
                                     scale=sr, bias=br)
            if RSP < HW:
                fnb = work.tile([128, HW - RSP], f16, tag="fnb")
                nc.vector.tensor_scalar(fnb[:], FN[:, RSP:HW], sr, br,
                                        OP.mult, OP.add)
                nc.vector.tensor_tensor(rdn[:, RSP:HW], fnb[:], fnb[:],
                                        OP.mult)
            # u = theta/2pi - am/2pi; round(u) via the int16 output
            # conversion (rounds to nearest), then d = theta/2pi - round(u)
            t2 = work.tile([128, HW], i16, tag="t2")
            d = work.tile([128, HW], f16, tag="d")
            if it == 0:
                nc.vector.tensor_scalar(t2[:, 0:HH], V[:, 0:HH], a2,
                                        None, OP.add)
                nc.vector.tensor_tensor(d[:, 0:HH], V[:, 0:HH],
                                        t2[:, 0:HH], OP.subtract)
                nc.vector.tensor_scalar(t2[:, HH:HW], V[:, HH:HW], a2,
                                        None, OP.add)
                nc.vector.tensor_tensor(d[:, HH:HW], V[:, HH:HW],
                                        t2[:, HH:HW], OP.subtract)
            else:
                nc.vector.tensor_scalar(t2[:], V, a2, None, OP.add)
                nc.vector.tensor_tensor(d[:], V, t2[:], OP.subtract)
            # sqa = (s2*d + b2)^2 = inv2av * wrap(theta-am)^2
            sqa = work.tile([128, HW], f16, tag="sqa")
            tt = work.tile([128, HW], f16, tag="tt")
            if it >= CL - 2:
                nc.scalar.activation(sqa[:, 0:HH], d[:, 0:HH], AF.Square,
                                     scale=s2, bias=b2)
                nc.vector.tensor_tensor(tt[:, 0:HH], sqa[:, 0:HH],
                                        rdn[:, 0:HH], OP.add)
                nc.scalar.activation(sqa[:, HH:HW], d[:, HH:HW], AF.Square,
                                     scale=s2, bias=b2)
                nc.vector.tensor_tensor(tt[:, HH:HW], sqa[:, HH:HW],
                                        rdn[:, HH:HW], OP.add)
            else:
                nc.scalar.activation(sqa[:], d[:], AF.Square,
                                     scale=s2, bias=b2)
                # tt = sqa + rdn ; e = x * exp(-tt) = exp(-tt + ln x)
                nc.vector.tensor_tensor(tt[:], sqa[:], rdn[:], OP.add)
            e = work.tile([128, HW], f16, tag="e", bufs=3)
            if it >= CL - 2:
                # tail channels: half-size exp lets the PE/DVE backend start
                # on the first half while ACT finishes the second
                nc.scalar.activation(e[:, 0:HH], tt[:, 0:HH], AF.Exp,
                                     scale=-1.0, bias=lx)
                nc.scalar.activation(e[:, HH:HW], tt[:, HH:HW], AF.Exp,
                                     scale=-1.0, bias=lx)
            else:
                nc.scalar.activation(e[:], tt[:], AF.Exp, scale=-1.0, bias=lx)
            # transposes (PE-only; reduces emitted one channel later)
            pss = []
            for g in range(NGRP):
                ps = psum.tile([128, 1024], f16, tag="ps")
                for l in range(8):
                    k = g * 8 + l
                    nc.tensor.transpose(
                        ps[:, l * 128 : (l + 1) * 128],
                        e[:, k * 128 : (k + 1) * 128],
                        ID,
                    )
                pss.append(ps)
            return pss

        def emit_backend(it, pss):
            o_t = outp.tile([128, NBLK, 2], f16, tag="o_t")
            for g in range(NGRP):
                red_in = pss[g][:, :].rearrange("p (l r o) -> p l r o",
                                                l=8, r=2, o=64)
                nc.vector.tensor_reduce(
                    o_t[:, g * 8 : (g + 1) * 8, :], red_in,
                    axis=AX.X, op=OP.max,
                )
            # center fixup + fp32 convert into the output accumulator
            fx = FX[:, it * 64 : (it + 1) * 64].rearrange(
                "p (blk b) -> p blk b", blk=NBLK, b=2)
            nc.vector.tensor_tensor(OALL[:, it, :, :], o_t[:, :, :], fx,
                                    OP.max)
            nc.sync.dma_start(out_d[:, it * 64 : (it + 1) * 64],
                              OALL[:, it, :, :])

        prev = None
        for it in range(CL):
            pss = emit_elementwise(it)
            if prev is not None:
                emit_backend(it - 1, prev)
            prev = pss
        emit_backend(CL - 1, prev)
    nc.finalize()
    return nc


def _host_scalars(x, radius_mean, angle_mean, radius_std, angle_std):
    """Per-core host tables.  partition = b*64 + o."""
    inv2rv = 1.0 / (2.0 * (radius_std.astype(np.float64) ** 2 + 0.01))   # [C]
    inv2av = 1.0 / (2.0 * (angle_std.astype(np.float64) ** 2 + 0.0001))  # [C]
    rm = np.abs(radius_mean.astype(np.float64)).reshape(B, C, NO)
    am = angle_mean.astype(np.float64).reshape(B, C, NO)
    xx = x.astype(np.float64).reshape(B, C, NO)
    lxx = np.log(np.maximum(xx, 1e-30))
    per_core = []
    for k in range(NCORES):
        cs = np.arange(k * CL, (k + 1) * CL)
        sc32 = np.zeros((128, 6 * CL))
        fxf = np.zeros((128, CL * 64))
        for itc, c in enumerate(cs):
            s2 = TWO_PI * np.sqrt(inv2av[c])
            srt = np.sqrt(inv2rv[c])
            for b in range(B):
                p = slice(b * NO, (b + 1) * NO)
                sc32[p, 0 * CL + itc] = -am[b, c] / TWO_PI
                sc32[p, 1 * CL + itc] = s2
                sc32[p, 2 * CL + itc] = -s2 * am[b, c] / TWO_PI
                sc32[p, 3 * CL + itc] = srt
                sc32[p, 4 * CL + itc] = -rm[b, c] * srt
                sc32[p, 5 * CL + itc] = lxx[b, c]
                # center fixup: value at pixel (8i, 8j) from origin (i,j)
                cc = xx[b, c] * np.exp(-(rm[b, c] ** 2) * inv2rv[c])
                for o in range(NO):
                    i, j = o // NW, o % NW
                    fxf[8 * j, itc * 64 + (4 * i) * 2 + b] = cc[o]
        per_core.append((
            np.ascontiguousarray(sc32.astype(np.float32)),
            np.ascontiguousarray(fxf.astype(np.float16)),
        ))
    return per_core


def _make_in_maps(x, radius_mean, angle_mean, radius_std, angle_std):
    if "nc" not in _built:
        _built["nc"] = _build_bass()
        _built["fields"] = _host_fields()
    v, fn = _built["fields"]
    c16a = np.ascontiguousarray(np.concatenate([v, fn], axis=1))
    ident = np.eye(128, dtype=np.float16)
    sc = _host_scalars(x, radius_mean, angle_mean, radius_std, angle_std)
    in_maps = []
    for k in range(NCORES):
        sc32, fxf = sc[k]
        c16b = np.ascontiguousarray(np.concatenate([ident, fxf], axis=1))
        in_maps.append({"c16a": c16a, "c16b": c16b, "c32": sc32})
    return in_maps


def kernel(x, radius_mean, angle_mean, radius_std, angle_std):
    from concourse.bass_utils import run_bass_kernel_spmd

    in_maps = _make_in_maps(x, radius_mean, angle_mean, radius_std, angle_std)
    nc = _built["nc"]
    res = run_bass_kernel_spmd(nc, in_maps, core_ids=list(range(NCORES)))
    out = np.empty((B, C, H, W), dtype=np.float32)
    for k in range(NCORES):
        r = res.results[k]["out"].reshape(128, CL, NBLK, 2)
        r = r.transpose(3, 1, 2, 0).reshape(B, CL, H, W)
        out[:, k * CL : (k + 1) * CL] = r
    return out
